# revision 5
# baseline (speedup 1.0000x reference)
"""Multi-head attention (B=1, S=4096, H=12, d_head=64, d_model=768) on 8
Trainium2 NeuronCores — v3.

All-bf16 datapath (fp8 injects ~1% output error: per-element quantization
noise propagates 1:1 through softmax averaging — signal and noise shrink
together). Wins over the v1 baseline:
  - Bias elimination: bk dropped (adds a per-query constant to scores =
    softmax invariant), bv folded into bo on the host (attention weights
    sum to 1), bq fused into the Q PSUM eviction (per-partition activation
    bias), bo via one rank-1 matmul per output tile.
  - Softmax exp split between the Scalar engine (true Exp with the 1/8
    scale and a -2 shift fused) and the Vector engine (Schraudolph int16
    bit-trick straight from PSUM, debiased; bits read back as bf16).
    The -2 shift cancels in normalization.
  - Everything AllGathered (no redundant full-sequence local projections):
    chunk 0 = pair 0 lands while Q/K/V shard projections keep the PE busy.
  - V is AllGathered in [128x64] head-blocks so attention-time V loads are
    single contiguous descriptors.
  - Normalization: Z scattered to [128, 8] so the DVE reciprocal uses all
    lanes, DRAM stride-0 broadcast, multiply on GpSimd.
  - Output projection in two 384-wide chunks with the scalar engine doing
    final evictions.
"""

import math

import numpy as np


def _ensure_paths():
    try:
        import concourse  # noqa: F401
    except ImportError:
        import sys

        for p in ("/opt/trn_rl_repo", "/root/.axon_site/_ro/trn_rl_repo"):
            if p not in sys.path:
                sys.path.append(p)


_ensure_paths()

N_HEADS = 12
D_MODEL = 768
DH = 64
B = 1
S = 4096
N_CORES = 8
P = 128
LOG2E = 1.4426950408889634
ESHIFT = 2.0
SCHRAUD_DEBIAS = 2.0 - 1.0 / math.log(2.0) - 0.5


def install_ntff_hook():
    import sys
    import types

    try:
        from antenv.axon_hooks import get_axon_ntff_profile_hook  # noqa: F401

        return True
    except ImportError:
        pass
    try:
        import antenv
        from trn_agent_boot.trn_boot import _ntff_profile_via_ctypes

        hook = _ntff_profile_via_ctypes("/opt/axon/libaxon_pjrt.so")
        if hook is None:
            return False
        mod = types.ModuleType("antenv.axon_hooks")
        mod._hook = hook

        def set_axon_ntff_profile_hook(h):
            mod._hook = h

        def get_axon_ntff_profile_hook():
            return mod._hook

        mod.set_axon_ntff_profile_hook = set_axon_ntff_profile_hook
        mod.get_axon_ntff_profile_hook = get_axon_ntff_profile_hook
        sys.modules["antenv.axon_hooks"] = mod
        antenv.axon_hooks = mod
        return True
    except Exception:
        return False


def build_attention_nc(s_total=S, n_cores=N_CORES, n_heads=N_HEADS, dh=DH,
                       d_model=D_MODEL, scalar_frac=11):
    """scalar_frac/16 of exp k-tiles run on Scalar, rest on DVE."""
    import concourse.bass as bass  # noqa: F401
    import concourse.mybir as mybir
    import concourse.tile as tile
    from concourse import bacc

    dt = mybir.dt
    BF = dt.bfloat16
    F32 = dt.float32
    I16 = dt.int16
    EXP = mybir.ActivationFunctionType.Exp
    IDN = mybir.ActivationFunctionType.Identity
    CPY = mybir.ActivationFunctionType.Copy
    MUL = mybir.AluOpType.mult
    ADD = mybir.AluOpType.add

    HD = n_heads * dh
    assert HD == d_model
    SQ = s_total // n_cores       # 512
    NK = d_model // P             # 6 contraction tiles
    NPAIR = n_heads // 2          # 6
    NT = s_total // P             # 32 key tiles per pair
    NSQT = SQ // P                # 4
    VW = 160                      # vt row: [A 64 | 1 | pad | B 64 | 1 | pad]
    NVS = 8
    CHW = 512
    NCH = s_total // CHW
    e_scale = 1.0 / 8.0
    d_scale = P * LOG2E / 8.0     # schraudolph for bf16 bits
    d_bias = P * (127.0 - ESHIFT * LOG2E - SCHRAUD_DEBIAS)

    nc = bacc.Bacc("TRN2", target_bir_lowering=False, debug=False,
                   num_devices=n_cores)

    xt = nc.dram_tensor("xt", [P, NK, SQ], BF, kind="ExternalInput")
    xf = nc.dram_tensor("xf", [P, NK, s_total], BF, kind="ExternalInput")
    wq = nc.dram_tensor("wq", [P, NK, HD], BF, kind="ExternalInput")
    wk = nc.dram_tensor("wk", [P, NK, HD], BF, kind="ExternalInput")
    wv = nc.dram_tensor("wv", [P, NK, HD], BF, kind="ExternalInput")
    wo = nc.dram_tensor("wo", [P, NPAIR, d_model], BF, kind="ExternalInput")
    bqc = nc.dram_tensor("bqc", [P, NPAIR], F32, kind="ExternalInput")
    bo = nc.dram_tensor("bo", [1, d_model], BF, kind="ExternalInput")
    out = nc.dram_tensor("out", [SQ, d_model], F32, kind="ExternalOutput")

    def exp_on_scalar(idx):
        return (idx * scalar_frac) % 16 < scalar_frac

    with tile.TileContext(nc) as tc:
        from contextlib import ExitStack

        with ExitStack() as ctx:
            const = ctx.enter_context(tc.tile_pool(name="const", bufs=1))
            io = ctx.enter_context(tc.tile_pool(name="io", bufs=3))
            vio = ctx.enter_context(tc.tile_pool(name="vio", bufs=6))
            ps = ctx.enter_context(
                tc.tile_pool(name="ps", bufs=2, space="PSUM"))
            psY = ctx.enter_context(
                tc.tile_pool(name="psY", bufs=2, space="PSUM"))
            dram = ctx.enter_context(
                tc.tile_pool(name="dram", bufs=1, space="DRAM"))

            # ---------------- constants & weights ----------------
            ones_bf = const.tile([1, P], BF, tag="ones")
            nc.vector.memset(ones_bf[:], 1.0)
            biasm = const.tile([P, 1], F32, tag="biasm")
            nc.vector.memset(biasm[:], -ESHIFT)

            # producer-side V eviction slots with a baked ones column;
            # the ones ride through the AllGather inside each [128,65] block
            vev = const.tile([P, NVS, dh + 1], BF, tag="vev")
            for s in range(NVS):
                nc.vector.memset(vev[:, s, dh:dh + 1], 1.0)
            vev_i = [0]

            xt_sb = const.tile([P, NK, SQ], BF, tag="xt_sb")
            wq_sb = const.tile([P, NK, HD], BF, tag="wq_sb")
            wk_sb = const.tile([P, NK, HD], BF, tag="wk_sb")
            wv_sb = const.tile([P, NK, HD], BF, tag="wv_sb")
            nc.sync.dma_start(xt_sb[:], xt[:, :, :])
            nc.sync.dma_start(wk_sb[:], wk[:, :, :])
            nc.sync.dma_start(wv_sb[:], wv[:, :, :])
            nc.sync.dma_start(wq_sb[:], wq[:, :, :])
            wo_sb = const.tile([P, NPAIR, d_model], BF, tag="wo_sb")
            nc.sync.dma_start(wo_sb[:], wo[:, :, :])
            bq_sb = const.tile([P, NPAIR], F32, tag="bq_sb")
            nc.sync.dma_start(bq_sb[:], bqc[:, :])
            bo_sb = const.tile([1, d_model], BF, tag="bo_sb")
            nc.sync.dma_start(bo_sb[:], bo[:, :])

            qsb = const.tile([P, NPAIR, SQ], BF, tag="qsb")
            zbs_all = const.tile([P, NPAIR, SQ], F32, tag="zbs_all")
            y_sb = const.tile([P, NPAIR, SQ], BF, tag="y_sb")
            ynorm = const.tile([P, NPAIR, SQ], BF, tag="ynorm")

            kl = const.tile([P, NCH, CHW], BF, tag="kl")
            vlv = const.tile([P, NCH, 4, 2, dh + 1], BF, tag="vlv")
            for c in range(NCH):
                nc.vector.memset(vlv[:, c, :, :, dh:dh + 1], 1.0)

            scr = const.tile([1, 8], F32, tag="scr")
            nc.scalar.activation(scr[:], ones_bf[:, 0:8], EXP)

            # ---------------- DRAM collective tiles ----------------
            rg = [list(range(n_cores))]
            CH_PAIRS = [(1, 2), (3, 3)]
            kag, vag = {}, {}
            for ci, (p0, np_) in enumerate(CH_PAIRS):
                cw = np_ * P
                kb = dram.tile([cw, SQ], BF, tag=f"kb{ci}")
                ka = dram.tile([n_cores * cw, SQ], BF, tag=f"kag{ci}",
                               addr_space="Shared")
                kag[ci] = (kb, ka)
                vb = dram.tile([np_ * 2, P, NSQT * (dh + 1)], BF,
                               tag=f"vb{ci}")
                va = dram.tile([n_cores * np_ * 2, P, NSQT * (dh + 1)], BF,
                               tag=f"vag{ci}", addr_space="Shared")
                vag[ci] = (vb, va)
            pair2ch = {}
            for ci, (p0, np_) in enumerate(CH_PAIRS):
                for pl in range(np_):
                    pair2ch[p0 + pl] = (ci, pl)

            # ---------------- shard projections ----------------
            def proj_pair(w_sb, cs):
                pt = ps.tile([P, 2, SQ], F32, tag="sc")
                o = pt[:, 0, :]
                for k in range(NK):
                    nc.tensor.matmul(o, lhsT=w_sb[:, k, cs:cs + P],
                                     rhs=xt_sb[:, k, :],
                                     start=(k == 0), stop=(k == NK - 1))
                return pt

            def vproj_stile(s_, cs, width):
                pt = ps.tile([P, 2, SQ], F32, tag="sc")
                o = pt[:, 0, 0:width]
                rs = s_ * P
                for k in range(NK):
                    nc.tensor.matmul(
                        o, lhsT=xt_sb[:, k, rs:rs + P],
                        rhs=wv_sb[:, k, cs:cs + width],
                        start=(k == 0), stop=(k == NK - 1))
                return pt

            def emit_chunk(ci):
                p0, np_ = CH_PAIRS[ci]
                kb, ka = kag[ci]
                for pl in range(np_):
                    pt = proj_pair(wk_sb, (p0 + pl) * P)
                    k16 = io.tile([P, SQ], BF, tag="k16")
                    nc.scalar.activation(k16[:], pt[:, 0, :], CPY)
                    nc.scalar.dma_start(kb[pl * P:(pl + 1) * P, :], k16[:])
                nc.gpsimd.collective_compute(
                    "AllGather", mybir.AluOpType.bypass, replica_groups=rg,
                    ins=[kb.opt()], outs=[ka.opt()])
                vb, va = vag[ci]
                cw = np_ * P
                for s_ in range(NSQT):
                    pt = vproj_stile(s_, p0 * P, cw)
                    for h2 in range(np_ * 2):
                        sl = vev_i[0] % NVS
                        vev_i[0] += 1
                        if h2 % 2 == 0:
                            nc.scalar.activation(
                                vev[:, sl, 0:dh],
                                pt[:, 0, h2 * dh:(h2 + 1) * dh], CPY)
                        else:
                            nc.vector.tensor_copy(
                                vev[:, sl, 0:dh],
                                pt[:, 0, h2 * dh:(h2 + 1) * dh])
                        nc.scalar.dma_start(
                            vb[h2, :, s_ * (dh + 1):(s_ + 1) * (dh + 1)],
                            vev[:, sl, :])
                nc.gpsimd.collective_compute(
                    "AllGather", mybir.AluOpType.bypass, replica_groups=rg,
                    ins=[vb.opt()], outs=[va.opt()])

            emit_chunk(0)
            for p in range(NPAIR):
                pt = proj_pair(wq_sb, p * P)
                nc.scalar.activation(qsb[:, p, :], pt[:, 0, :], IDN,
                                     bias=bq_sb[:, p:p + 1], scale=1.0)
            emit_chunk(1)

            # ---------------- attention ----------------
            cur = {}

            def new_pair_accum():
                yA_t = psY.tile([dh + 1, SQ], F32, tag="yA")
                yB_t = psY.tile([dh + 1, SQ], F32, tag="yB")
                cur["yA"], cur["yB"] = yA_t, yB_t
            exp_idx = [0]

            def emit_attnv(pend, last):
                at16, vst, tl, t = pend
                for h in range(2):
                    yT = cur["yA"] if h == 0 else cur["yB"]
                    nc.tensor.matmul(
                        yT[:], lhsT=vst[:, h, tl, :],
                        rhs=at16[:, h, :],
                        start=(t == 0), stop=last)

            def finish_pair(p):
                yA, yB = cur["yA"], cur["yB"]
                zrt = io.tile([dh + 1, 2, SQ], F32, tag="zrt")
                nc.vector.tensor_copy(zrt[dh:dh + 1, 0, :], yA[dh:dh + 1, :])
                nc.vector.tensor_copy(zrt[dh:dh + 1, 1, :], yB[dh:dh + 1, :])
                zt = io.tile([P, 8], F32, tag="zt")
                nc.scalar.dma_start(zt[:], zrt[dh:dh + 1, :, :])
                zr = io.tile([P, 8], F32, tag="zr")
                nc.vector.reciprocal(zr[:], zt[:])
                zd = dram.tile([2, SQ], F32, tag=f"zd{p}")
                nc.scalar.dma_start(zd[:, :], zr[:])
                nc.vector.tensor_copy(y_sb[0:dh, p, :], yA[0:dh, :])
                ybt = io.tile([dh, SQ], BF, tag="ybt")
                nc.vector.tensor_copy(ybt[:], yB[0:dh, :])
                nc.scalar.dma_start(y_sb[dh:2 * dh, p, :], ybt[:])
                nc.scalar.dma_start(zbs_all[0:dh, p, :],
                                    zd[0:1, :].to_broadcast((dh, SQ)))
                nc.scalar.dma_start(zbs_all[dh:2 * dh, p, :],
                                    zd[1:2, :].to_broadcast((dh, SQ)))

            # ---- pair 0: local full-sequence K/V from xf ----
            new_pair_accum()
            pend = []
            for c8 in range(NCH):
                xfc = io.tile([P, NK, CHW], BF, tag="xfc")
                nc.sync.dma_start(xfc[:],
                                  xf[:, :, c8 * CHW:(c8 + 1) * CHW])
                pt = ps.tile([P, 2, SQ], F32, tag="sc")
                for k in range(NK):
                    nc.tensor.matmul(pt[:, 0, :], lhsT=wk_sb[:, k, 0:P],
                                     rhs=xfc[:, k, :],
                                     start=(k == 0), stop=(k == NK - 1))
                nc.vector.tensor_copy(kl[:, c8, :], pt[:, 0, :])
                for tt in range(4):
                    pv = ps.tile([P, 2, SQ], F32, tag="sc")
                    o = pv[:, 0, 0:P]
                    for k in range(NK):
                        nc.tensor.matmul(
                            o, lhsT=xfc[:, k, tt * P:(tt + 1) * P],
                            rhs=wv_sb[:, k, 0:P],
                            start=(k == 0), stop=(k == NK - 1))
                    nc.scalar.activation(vlv[:, c8, tt, 0, 0:dh],
                                         pv[:, 0, 0:dh], CPY)
                    nc.vector.tensor_copy(vlv[:, c8, tt, 1, 0:dh],
                                          pv[:, 0, dh:2 * dh])
                for tt in range(4):
                    t = c8 * 4 + tt
                    sc = ps.tile([P, 2, SQ], F32, tag="sc")
                    for h in range(2):
                        nc.tensor.matmul(
                            sc[:, h, :],
                            lhsT=kl[h * dh:(h + 1) * dh, c8,
                                    tt * P:(tt + 1) * P],
                            rhs=qsb[h * dh:(h + 1) * dh, 0, :],
                            start=True, stop=True,
                            tile_position=(h * dh, 0))
                    if len(pend) >= 2:
                        at16o, vsto, tlo, to = pend.pop(0)
                        for h in range(2):
                            yT = cur["yA"] if h == 0 else cur["yB"]
                            nc.tensor.matmul(
                                yT[:], lhsT=vlv[:, to // 4, to % 4, h, :],
                                rhs=at16o[:, h, :],
                                start=(to == 0), stop=False)
                    at16 = vio.tile([P, 2, SQ], BF, tag="at16")
                    i = exp_idx[0]
                    exp_idx[0] += 1
                    if exp_on_scalar(i):
                        nc.scalar.activation(at16[:], sc[:], EXP,
                                             bias=biasm[:], scale=e_scale)
                    else:
                        nc.vector.tensor_scalar(at16[:].bitcast(I16), sc[:],
                                                d_scale, d_bias, MUL, ADD)
                    pend.append((at16, None, tt, t))
            for pi, (at16o, _, _, to) in enumerate(pend):
                for h in range(2):
                    yT = cur["yA"] if h == 0 else cur["yB"]
                    nc.tensor.matmul(
                        yT[:], lhsT=vlv[:, to // 4, to % 4, h, :],
                        rhs=at16o[:, h, :],
                        start=(to == 0), stop=(pi == len(pend) - 1))
            pend = []
            finish_pair(0)
            ynorm_q = [0]

            for p in range(1, NPAIR):
                new_pair_accum()
                ci, pl = pair2ch[p]
                cw = CH_PAIRS[ci][1] * P
                n2 = CH_PAIRS[ci][1] * 2
                ka = kag[ci][1]
                va = vag[ci][1]
                pend = []
                for r in range(n_cores):
                    ktp = vio.tile([P, SQ], BF, tag="ktp")
                    nc.gpsimd.dma_start(
                        ktp[:], ka[r * cw + pl * P:r * cw + (pl + 1) * P, :])
                    # all of this rank's V for the pair: one fat DMA into
                    # contiguous staging (fat 520B descriptors)
                    vst = vio.tile([P, 2, NSQT, dh + 1], BF, tag="vst")
                    nc.gpsimd.dma_start(
                        vst[:],
                        va[r * n2 + 2 * pl:r * n2 + 2 * pl + 2, :, :]
                        .transpose([1, 0, 2]))
                    for tr in range(NSQT):
                        t = r * NSQT + tr
                        sc = ps.tile([P, 2, SQ], F32, tag="sc")
                        for h in range(2):
                            nc.tensor.matmul(
                                sc[:, h, :],
                                lhsT=ktp[h * dh:(h + 1) * dh,
                                         tr * P:(tr + 1) * P],
                                rhs=qsb[h * dh:(h + 1) * dh, p, :],
                                start=True, stop=True,
                                tile_position=(h * dh, 0))
                        if len(pend) >= 2:
                            emit_attnv(pend.pop(0), False)
                        at16 = vio.tile([P, 2, SQ], BF, tag="at16")
                        i = exp_idx[0]
                        exp_idx[0] += 1
                        if exp_on_scalar(i):
                            nc.scalar.activation(at16[:], sc[:], EXP,
                                                 bias=biasm[:],
                                                 scale=e_scale)
                        else:
                            nc.vector.tensor_scalar(
                                at16[:].bitcast(I16), sc[:],
                                d_scale, d_bias, MUL, ADD)
                        pend.append((at16, vst, tr, t))
                while len(pend) > 1:
                    emit_attnv(pend.pop(0), False)
                emit_attnv(pend.pop(0), True)
                finish_pair(p)

            # remaining normalization multiplies, split across the two
            # idle-by-now engines so the output projection starts sooner
            for p in range(ynorm_q[0], NPAIR):
                eng = nc.gpsimd if p % 2 == 0 else nc.vector
                eng.tensor_mul(out=ynorm[:, p, :],
                               in0=y_sb[:, p, :],
                               in1=zbs_all[:, p, :])

            # ---------------- output projection ----------------
            DMC = 384
            for s_ in range(NSQT):
                rs = s_ * P
                pt = ps.tile([P, 2, SQ], F32, tag="sc")
                for ch in range(2):
                    o = pt[:, ch, 0:DMC]
                    c0 = ch * DMC
                    for p in range(NPAIR):
                        nc.tensor.matmul(o, lhsT=ynorm[:, p, rs:rs + P],
                                         rhs=wo_sb[:, p, c0:c0 + DMC],
                                         start=(p == 0), stop=False)
                    nc.tensor.matmul(o, lhsT=ones_bf[:, 0:P],
                                     rhs=bo_sb[:, c0:c0 + DMC],
                                     start=False, stop=True)
                osb = io.tile([P, 2, DMC], F32, tag="osb")
                nc.scalar.activation(osb[:], pt[:, :, 0:DMC], CPY)
                nc.sync.dma_start(out[rs:rs + P, :], osb[:])

    nc.compile()
    return nc


# ---------------------------------------------------------------------------
_CACHE = {}


def _get_nc():
    if "nc" not in _CACHE:
        _CACHE["nc"] = build_attention_nc()
    return _CACHE["nc"]


def _fold6(a):
    """[768, N] -> [128, 6, N]: row = 128k + p."""
    n = a.shape[1]
    return np.ascontiguousarray(a.reshape(6, 128, n).transpose(1, 0, 2))


def make_in_maps(x, Wq, bq, Wk, bk, Wv, bv, Wo, bo, n_cores=N_CORES):
    import ml_dtypes

    bf = ml_dtypes.bfloat16
    sq = x.shape[1] // n_cores
    x2 = np.asarray(x, np.float32).reshape(x.shape[1], D_MODEL)
    xT = x2.T

    Wo_f = np.asarray(Wo, np.float32)
    shared = {
        "wq": _fold6(np.asarray(Wq, np.float32)).astype(bf),
        "wk": _fold6(np.asarray(Wk, np.float32)).astype(bf),
        "wv": _fold6(np.asarray(Wv, np.float32)).astype(bf),
        "wo": np.ascontiguousarray(
            Wo_f.reshape(N_HEADS // 2, P, D_MODEL).transpose(1, 0, 2)
        ).astype(bf),
        "bqc": np.ascontiguousarray(
            np.asarray(bq, np.float32).reshape(N_HEADS // 2, P).T),
        "bo": (np.asarray(bo, np.float32)
               + np.asarray(bv, np.float32) @ Wo_f).reshape(1, -1).astype(bf),
    }
    shared["xf"] = _fold6(xT).astype(bf)
    in_maps = []
    for c in range(n_cores):
        xtc = _fold6(
            np.ascontiguousarray(xT[:, c * sq:(c + 1) * sq])).astype(bf)
        in_maps.append({"xt": xtc, **shared})
    return in_maps


def kernel(x, Wq, bq, Wk, bk, Wv, bv, Wo, bo):
    from concourse.bass_utils import run_bass_kernel_spmd

    nc = _get_nc()
    in_maps = make_in_maps(x, Wq, bq, Wk, bk, Wv, bv, Wo, bo)
    res = run_bass_kernel_spmd(nc, in_maps, core_ids=list(range(N_CORES)))
    out = np.concatenate([res.results[c]["out"] for c in range(N_CORES)],
                         axis=0)
    return out.reshape(B, S, D_MODEL).astype(np.float32)


# revision 6
# speedup vs baseline: 1.0075x; 1.0075x over previous
"""Multi-head attention (B=1, S=4096, H=12, d_head=64, d_model=768) on 8
Trainium2 NeuronCores — v3.

All-bf16 datapath (fp8 injects ~1% output error: per-element quantization
noise propagates 1:1 through softmax averaging — signal and noise shrink
together). Wins over the v1 baseline:
  - Bias elimination: bk dropped (adds a per-query constant to scores =
    softmax invariant), bv folded into bo on the host (attention weights
    sum to 1), bq fused into the Q PSUM eviction (per-partition activation
    bias), bo via one rank-1 matmul per output tile.
  - Softmax exp split between the Scalar engine (true Exp with the 1/8
    scale and a -2 shift fused) and the Vector engine (Schraudolph int16
    bit-trick straight from PSUM, debiased; bits read back as bf16).
    The -2 shift cancels in normalization.
  - Everything AllGathered (no redundant full-sequence local projections):
    chunk 0 = pair 0 lands while Q/K/V shard projections keep the PE busy.
  - V is AllGathered in [128x64] head-blocks so attention-time V loads are
    single contiguous descriptors.
  - Normalization: Z scattered to [128, 8] so the DVE reciprocal uses all
    lanes, DRAM stride-0 broadcast, multiply on GpSimd.
  - Output projection in two 384-wide chunks with the scalar engine doing
    final evictions.
"""

import math

import numpy as np


def _ensure_paths():
    try:
        import concourse  # noqa: F401
    except ImportError:
        import sys

        for p in ("/opt/trn_rl_repo", "/root/.axon_site/_ro/trn_rl_repo"):
            if p not in sys.path:
                sys.path.append(p)


_ensure_paths()

N_HEADS = 12
D_MODEL = 768
DH = 64
B = 1
S = 4096
N_CORES = 8
P = 128
LOG2E = 1.4426950408889634
ESHIFT = 2.0
SCHRAUD_DEBIAS = 2.0 - 1.0 / math.log(2.0) - 0.5


def install_ntff_hook():
    import sys
    import types

    try:
        from antenv.axon_hooks import get_axon_ntff_profile_hook  # noqa: F401

        return True
    except ImportError:
        pass
    try:
        import antenv
        from trn_agent_boot.trn_boot import _ntff_profile_via_ctypes

        hook = _ntff_profile_via_ctypes("/opt/axon/libaxon_pjrt.so")
        if hook is None:
            return False
        mod = types.ModuleType("antenv.axon_hooks")
        mod._hook = hook

        def set_axon_ntff_profile_hook(h):
            mod._hook = h

        def get_axon_ntff_profile_hook():
            return mod._hook

        mod.set_axon_ntff_profile_hook = set_axon_ntff_profile_hook
        mod.get_axon_ntff_profile_hook = get_axon_ntff_profile_hook
        sys.modules["antenv.axon_hooks"] = mod
        antenv.axon_hooks = mod
        return True
    except Exception:
        return False


def build_attention_nc(s_total=S, n_cores=N_CORES, n_heads=N_HEADS, dh=DH,
                       d_model=D_MODEL, scalar_frac=11):
    """scalar_frac/16 of exp k-tiles run on Scalar, rest on DVE."""
    import concourse.bass as bass  # noqa: F401
    import concourse.mybir as mybir
    import concourse.tile as tile
    from concourse import bacc

    dt = mybir.dt
    BF = dt.bfloat16
    F32 = dt.float32
    I16 = dt.int16
    EXP = mybir.ActivationFunctionType.Exp
    IDN = mybir.ActivationFunctionType.Identity
    CPY = mybir.ActivationFunctionType.Copy
    MUL = mybir.AluOpType.mult
    ADD = mybir.AluOpType.add

    HD = n_heads * dh
    assert HD == d_model
    SQ = s_total // n_cores       # 512
    NK = d_model // P             # 6 contraction tiles
    NPAIR = n_heads // 2          # 6
    NT = s_total // P             # 32 key tiles per pair
    NSQT = SQ // P                # 4
    VW = 160                      # vt row: [A 64 | 1 | pad | B 64 | 1 | pad]
    NVS = 8
    CHW = 512
    NCH = s_total // CHW
    e_scale = 1.0 / 8.0
    d_scale = P * LOG2E / 8.0     # schraudolph for bf16 bits
    d_bias = P * (127.0 - ESHIFT * LOG2E - SCHRAUD_DEBIAS)

    nc = bacc.Bacc("TRN2", target_bir_lowering=False, debug=False,
                   num_devices=n_cores)

    xt = nc.dram_tensor("xt", [P, NK, SQ], BF, kind="ExternalInput")
    xf = nc.dram_tensor("xf", [P, NK, s_total], BF, kind="ExternalInput")
    wq = nc.dram_tensor("wq", [P, NK, HD], BF, kind="ExternalInput")
    wk = nc.dram_tensor("wk", [P, NK, HD], BF, kind="ExternalInput")
    wv = nc.dram_tensor("wv", [P, NK, HD], BF, kind="ExternalInput")
    wo = nc.dram_tensor("wo", [P, NPAIR, d_model], BF, kind="ExternalInput")
    bqc = nc.dram_tensor("bqc", [P, NPAIR], F32, kind="ExternalInput")
    bo = nc.dram_tensor("bo", [1, d_model], BF, kind="ExternalInput")
    out = nc.dram_tensor("out", [SQ, d_model], F32, kind="ExternalOutput")

    def exp_on_scalar(idx):
        return (idx * scalar_frac) % 16 < scalar_frac

    with tile.TileContext(nc) as tc:
        from contextlib import ExitStack

        with ExitStack() as ctx:
            const = ctx.enter_context(tc.tile_pool(name="const", bufs=1))
            io = ctx.enter_context(tc.tile_pool(name="io", bufs=3))
            vio = ctx.enter_context(tc.tile_pool(name="vio", bufs=6))
            ps = ctx.enter_context(
                tc.tile_pool(name="ps", bufs=3, space="PSUM"))
            psY = ctx.enter_context(
                tc.tile_pool(name="psY", bufs=1, space="PSUM"))
            dram = ctx.enter_context(
                tc.tile_pool(name="dram", bufs=1, space="DRAM"))

            # ---------------- constants & weights ----------------
            ones_bf = const.tile([1, P], BF, tag="ones")
            nc.vector.memset(ones_bf[:], 1.0)
            biasm = const.tile([P, 1], F32, tag="biasm")
            nc.vector.memset(biasm[:], -ESHIFT)

            # producer-side V eviction slots with a baked ones column;
            # the ones ride through the AllGather inside each [128,65] block
            vev = const.tile([P, NVS, dh + 1], BF, tag="vev")
            for s in range(NVS):
                nc.vector.memset(vev[:, s, dh:dh + 1], 1.0)
            vev_i = [0]

            xt_sb = const.tile([P, NK, SQ], BF, tag="xt_sb")
            wq_sb = const.tile([P, NK, HD], BF, tag="wq_sb")
            wk_sb = const.tile([P, NK, HD], BF, tag="wk_sb")
            wv_sb = const.tile([P, NK, HD], BF, tag="wv_sb")
            nc.sync.dma_start(xt_sb[:], xt[:, :, :])
            nc.sync.dma_start(wk_sb[:], wk[:, :, :])
            nc.sync.dma_start(wv_sb[:], wv[:, :, :])
            nc.sync.dma_start(wq_sb[:], wq[:, :, :])
            wo_sb = const.tile([P, NPAIR, d_model], BF, tag="wo_sb")
            nc.sync.dma_start(wo_sb[:], wo[:, :, :])
            bq_sb = const.tile([P, NPAIR], F32, tag="bq_sb")
            nc.sync.dma_start(bq_sb[:], bqc[:, :])
            bo_sb = const.tile([1, d_model], BF, tag="bo_sb")
            nc.sync.dma_start(bo_sb[:], bo[:, :])

            qsb = const.tile([P, NPAIR, SQ], BF, tag="qsb")
            zbs_all = const.tile([P, NPAIR, SQ], F32, tag="zbs_all")
            y_sb = const.tile([P, NPAIR, SQ], BF, tag="y_sb")
            ynorm = const.tile([P, NPAIR, SQ], BF, tag="ynorm")

            kl = const.tile([P, NCH, CHW], BF, tag="kl")
            vlv = const.tile([P, NCH, 4, 2, dh + 1], BF, tag="vlv")
            for c in range(NCH):
                nc.vector.memset(vlv[:, c, :, :, dh:dh + 1], 1.0)

            scr = const.tile([1, 8], F32, tag="scr")
            nc.scalar.activation(scr[:], ones_bf[:, 0:8], EXP)

            # ---------------- DRAM collective tiles ----------------
            rg = [list(range(n_cores))]
            CH_PAIRS = [(1, 2), (3, 3)]
            kag, vag = {}, {}
            for ci, (p0, np_) in enumerate(CH_PAIRS):
                cw = np_ * P
                kb = dram.tile([cw, SQ], BF, tag=f"kb{ci}")
                ka = dram.tile([n_cores * cw, SQ], BF, tag=f"kag{ci}",
                               addr_space="Shared")
                kag[ci] = (kb, ka)
                vb = dram.tile([np_ * 2, P, NSQT * (dh + 1)], BF,
                               tag=f"vb{ci}")
                va = dram.tile([n_cores * np_ * 2, P, NSQT * (dh + 1)], BF,
                               tag=f"vag{ci}", addr_space="Shared")
                vag[ci] = (vb, va)
            pair2ch = {}
            for ci, (p0, np_) in enumerate(CH_PAIRS):
                for pl in range(np_):
                    pair2ch[p0 + pl] = (ci, pl)

            # ---------------- shard projections ----------------
            def proj_pair(w_sb, cs):
                pt = ps.tile([P, 2, SQ], F32, tag="sc")
                o = pt[:, 0, :]
                for k in range(NK):
                    nc.tensor.matmul(o, lhsT=w_sb[:, k, cs:cs + P],
                                     rhs=xt_sb[:, k, :],
                                     start=(k == 0), stop=(k == NK - 1))
                return pt

            def vproj_stile(s_, cs, width):
                pt = ps.tile([P, 2, SQ], F32, tag="sc")
                o = pt[:, 0, 0:width]
                rs = s_ * P
                for k in range(NK):
                    nc.tensor.matmul(
                        o, lhsT=xt_sb[:, k, rs:rs + P],
                        rhs=wv_sb[:, k, cs:cs + width],
                        start=(k == 0), stop=(k == NK - 1))
                return pt

            def emit_chunk(ci):
                p0, np_ = CH_PAIRS[ci]
                kb, ka = kag[ci]
                for pl in range(np_):
                    pt = proj_pair(wk_sb, (p0 + pl) * P)
                    k16 = io.tile([P, SQ], BF, tag="k16")
                    nc.scalar.activation(k16[:], pt[:, 0, :], CPY)
                    nc.scalar.dma_start(kb[pl * P:(pl + 1) * P, :], k16[:])
                nc.gpsimd.collective_compute(
                    "AllGather", mybir.AluOpType.bypass, replica_groups=rg,
                    ins=[kb.opt()], outs=[ka.opt()])
                vb, va = vag[ci]
                cw = np_ * P
                for s_ in range(NSQT):
                    pt = vproj_stile(s_, p0 * P, cw)
                    for h2 in range(np_ * 2):
                        sl = vev_i[0] % NVS
                        vev_i[0] += 1
                        if h2 % 2 == 0:
                            nc.scalar.activation(
                                vev[:, sl, 0:dh],
                                pt[:, 0, h2 * dh:(h2 + 1) * dh], CPY)
                        else:
                            nc.vector.tensor_copy(
                                vev[:, sl, 0:dh],
                                pt[:, 0, h2 * dh:(h2 + 1) * dh])
                        nc.scalar.dma_start(
                            vb[h2, :, s_ * (dh + 1):(s_ + 1) * (dh + 1)],
                            vev[:, sl, :])
                nc.gpsimd.collective_compute(
                    "AllGather", mybir.AluOpType.bypass, replica_groups=rg,
                    ins=[vb.opt()], outs=[va.opt()])

            emit_chunk(0)
            for p in range(NPAIR):
                pt = proj_pair(wq_sb, p * P)
                nc.scalar.activation(qsb[:, p, :], pt[:, 0, :], IDN,
                                     bias=bq_sb[:, p:p + 1], scale=1.0)
            emit_chunk(1)

            # ---------------- attention ----------------
            yA = psY.tile([dh + 1, SQ], F32, tag="yA")
            yB = psY.tile([dh + 1, SQ], F32, tag="yB")
            exp_idx = [0]

            def emit_attnv(pend, last):
                at16, vst, tl, t = pend
                for h in range(2):
                    yT = yA if h == 0 else yB
                    nc.tensor.matmul(
                        yT[:], lhsT=vst[:, h, tl, :],
                        rhs=at16[:, h, :],
                        start=(t == 0), stop=last)

            def finish_pair(p):
                zrt = io.tile([dh + 1, 2, SQ], F32, tag="zrt")
                nc.vector.tensor_copy(zrt[dh:dh + 1, 0, :], yA[dh:dh + 1, :])
                nc.vector.tensor_copy(zrt[dh:dh + 1, 1, :], yB[dh:dh + 1, :])
                zt = io.tile([P, 8], F32, tag="zt")
                nc.scalar.dma_start(zt[:], zrt[dh:dh + 1, :, :])
                zr = io.tile([P, 8], F32, tag="zr")
                nc.vector.reciprocal(zr[:], zt[:])
                zd = dram.tile([2, SQ], F32, tag=f"zd{p}")
                nc.scalar.dma_start(zd[:, :], zr[:])
                nc.vector.tensor_copy(y_sb[0:dh, p, :], yA[0:dh, :])
                ybt = io.tile([dh, SQ], BF, tag="ybt")
                nc.vector.tensor_copy(ybt[:], yB[0:dh, :])
                nc.scalar.dma_start(y_sb[dh:2 * dh, p, :], ybt[:])
                nc.scalar.dma_start(zbs_all[0:dh, p, :],
                                    zd[0:1, :].to_broadcast((dh, SQ)))
                nc.scalar.dma_start(zbs_all[dh:2 * dh, p, :],
                                    zd[1:2, :].to_broadcast((dh, SQ)))

            # ---- pair 0: local full-sequence K/V from xf ----
            pend = []
            for c8 in range(NCH):
                xfc = io.tile([P, NK, CHW], BF, tag="xfc")
                nc.sync.dma_start(xfc[:],
                                  xf[:, :, c8 * CHW:(c8 + 1) * CHW])
                pt = ps.tile([P, 2, SQ], F32, tag="sc")
                for k in range(NK):
                    nc.tensor.matmul(pt[:, 0, :], lhsT=wk_sb[:, k, 0:P],
                                     rhs=xfc[:, k, :],
                                     start=(k == 0), stop=(k == NK - 1))
                nc.vector.tensor_copy(kl[:, c8, :], pt[:, 0, :])
                for tt in range(4):
                    pv = ps.tile([P, 2, SQ], F32, tag="sc")
                    o = pv[:, 0, 0:P]
                    for k in range(NK):
                        nc.tensor.matmul(
                            o, lhsT=xfc[:, k, tt * P:(tt + 1) * P],
                            rhs=wv_sb[:, k, 0:P],
                            start=(k == 0), stop=(k == NK - 1))
                    nc.scalar.activation(vlv[:, c8, tt, 0, 0:dh],
                                         pv[:, 0, 0:dh], CPY)
                    nc.vector.tensor_copy(vlv[:, c8, tt, 1, 0:dh],
                                          pv[:, 0, dh:2 * dh])
                for tt in range(4):
                    t = c8 * 4 + tt
                    sc = ps.tile([P, 2, SQ], F32, tag="sc")
                    for h in range(2):
                        nc.tensor.matmul(
                            sc[:, h, :],
                            lhsT=kl[h * dh:(h + 1) * dh, c8,
                                    tt * P:(tt + 1) * P],
                            rhs=qsb[h * dh:(h + 1) * dh, 0, :],
                            start=True, stop=True,
                            tile_position=(h * dh, 0))
                    if len(pend) >= 2:
                        at16o, vsto, tlo, to = pend.pop(0)
                        for h in range(2):
                            yT = yA if h == 0 else yB
                            nc.tensor.matmul(
                                yT[:], lhsT=vlv[:, to // 4, to % 4, h, :],
                                rhs=at16o[:, h, :],
                                start=(to == 0), stop=False)
                    at16 = vio.tile([P, 2, SQ], BF, tag="at16")
                    i = exp_idx[0]
                    exp_idx[0] += 1
                    if exp_on_scalar(i):
                        nc.scalar.activation(at16[:], sc[:], EXP,
                                             bias=biasm[:], scale=e_scale)
                    else:
                        nc.vector.tensor_scalar(at16[:].bitcast(I16), sc[:],
                                                d_scale, d_bias, MUL, ADD)
                    pend.append((at16, None, tt, t))
            for pi, (at16o, _, _, to) in enumerate(pend):
                for h in range(2):
                    yT = yA if h == 0 else yB
                    nc.tensor.matmul(
                        yT[:], lhsT=vlv[:, to // 4, to % 4, h, :],
                        rhs=at16o[:, h, :],
                        start=(to == 0), stop=(pi == len(pend) - 1))
            pend = []
            finish_pair(0)
            ynorm_q = [0]

            for p in range(1, NPAIR):
                ci, pl = pair2ch[p]
                cw = CH_PAIRS[ci][1] * P
                n2 = CH_PAIRS[ci][1] * 2
                ka = kag[ci][1]
                va = vag[ci][1]
                pend = []
                for r in range(n_cores):
                    ktp = vio.tile([P, SQ], BF, tag="ktp")
                    nc.gpsimd.dma_start(
                        ktp[:], ka[r * cw + pl * P:r * cw + (pl + 1) * P, :])
                    # all of this rank's V for the pair: one fat DMA into
                    # contiguous staging (fat 520B descriptors)
                    vst = vio.tile([P, 2, NSQT, dh + 1], BF, tag="vst")
                    nc.gpsimd.dma_start(
                        vst[:],
                        va[r * n2 + 2 * pl:r * n2 + 2 * pl + 2, :, :]
                        .transpose([1, 0, 2]))
                    for tr in range(NSQT):
                        t = r * NSQT + tr
                        sc = ps.tile([P, 2, SQ], F32, tag="sc")
                        for h in range(2):
                            nc.tensor.matmul(
                                sc[:, h, :],
                                lhsT=ktp[h * dh:(h + 1) * dh,
                                         tr * P:(tr + 1) * P],
                                rhs=qsb[h * dh:(h + 1) * dh, p, :],
                                start=True, stop=True,
                                tile_position=(h * dh, 0))
                        if len(pend) >= 2:
                            emit_attnv(pend.pop(0), False)
                        at16 = vio.tile([P, 2, SQ], BF, tag="at16")
                        i = exp_idx[0]
                        exp_idx[0] += 1
                        if exp_on_scalar(i):
                            nc.scalar.activation(at16[:], sc[:], EXP,
                                                 bias=biasm[:],
                                                 scale=e_scale)
                        else:
                            nc.vector.tensor_scalar(
                                at16[:].bitcast(I16), sc[:],
                                d_scale, d_bias, MUL, ADD)
                        pend.append((at16, vst, tr, t))
                while len(pend) > 1:
                    emit_attnv(pend.pop(0), False)
                emit_attnv(pend.pop(0), True)
                finish_pair(p)

            # remaining normalization multiplies
            for p in range(ynorm_q[0], NPAIR):
                nc.gpsimd.tensor_mul(out=ynorm[:, p, :],
                                     in0=y_sb[:, p, :],
                                     in1=zbs_all[:, p, :])

            # ---------------- output projection ----------------
            DMC = 384
            for s_ in range(NSQT):
                rs = s_ * P
                pt = ps.tile([P, 2, SQ], F32, tag="sc")
                for ch in range(2):
                    o = pt[:, ch, 0:DMC]
                    c0 = ch * DMC
                    for p in range(NPAIR):
                        nc.tensor.matmul(o, lhsT=ynorm[:, p, rs:rs + P],
                                         rhs=wo_sb[:, p, c0:c0 + DMC],
                                         start=(p == 0), stop=False)
                    nc.tensor.matmul(o, lhsT=ones_bf[:, 0:P],
                                     rhs=bo_sb[:, c0:c0 + DMC],
                                     start=False, stop=True)
                osb = io.tile([P, 2, DMC], F32, tag="osb")
                nc.scalar.activation(osb[:], pt[:, :, 0:DMC], CPY)
                nc.sync.dma_start(out[rs:rs + P, :], osb[:])

    nc.compile()
    return nc


# ---------------------------------------------------------------------------
_CACHE = {}


def _get_nc():
    if "nc" not in _CACHE:
        _CACHE["nc"] = build_attention_nc()
    return _CACHE["nc"]


def _fold6(a):
    """[768, N] -> [128, 6, N]: row = 128k + p."""
    n = a.shape[1]
    return np.ascontiguousarray(a.reshape(6, 128, n).transpose(1, 0, 2))


def make_in_maps(x, Wq, bq, Wk, bk, Wv, bv, Wo, bo, n_cores=N_CORES):
    import ml_dtypes

    bf = ml_dtypes.bfloat16
    sq = x.shape[1] // n_cores
    x2 = np.asarray(x, np.float32).reshape(x.shape[1], D_MODEL)
    xT = x2.T

    Wo_f = np.asarray(Wo, np.float32)
    shared = {
        "wq": _fold6(np.asarray(Wq, np.float32)).astype(bf),
        "wk": _fold6(np.asarray(Wk, np.float32)).astype(bf),
        "wv": _fold6(np.asarray(Wv, np.float32)).astype(bf),
        "wo": np.ascontiguousarray(
            Wo_f.reshape(N_HEADS // 2, P, D_MODEL).transpose(1, 0, 2)
        ).astype(bf),
        "bqc": np.ascontiguousarray(
            np.asarray(bq, np.float32).reshape(N_HEADS // 2, P).T),
        "bo": (np.asarray(bo, np.float32)
               + np.asarray(bv, np.float32) @ Wo_f).reshape(1, -1).astype(bf),
    }
    shared["xf"] = _fold6(xT).astype(bf)
    in_maps = []
    for c in range(n_cores):
        xtc = _fold6(
            np.ascontiguousarray(xT[:, c * sq:(c + 1) * sq])).astype(bf)
        in_maps.append({"xt": xtc, **shared})
    return in_maps


def kernel(x, Wq, bq, Wk, bk, Wv, bv, Wo, bo):
    from concourse.bass_utils import run_bass_kernel_spmd

    nc = _get_nc()
    in_maps = make_in_maps(x, Wq, bq, Wk, bk, Wv, bv, Wo, bo)
    res = run_bass_kernel_spmd(nc, in_maps, core_ids=list(range(N_CORES)))
    out = np.concatenate([res.results[c]["out"] for c in range(N_CORES)],
                         axis=0)
    return out.reshape(B, S, D_MODEL).astype(np.float32)


# revision 7
# speedup vs baseline: 1.0076x; 1.0001x over previous
"""Multi-head attention (B=1, S=4096, H=12, d_head=64, d_model=768) on 8
Trainium2 NeuronCores — v3.

All-bf16 datapath (fp8 injects ~1% output error: per-element quantization
noise propagates 1:1 through softmax averaging — signal and noise shrink
together). Wins over the v1 baseline:
  - Bias elimination: bk dropped (adds a per-query constant to scores =
    softmax invariant), bv folded into bo on the host (attention weights
    sum to 1), bq fused into the Q PSUM eviction (per-partition activation
    bias), bo via one rank-1 matmul per output tile.
  - Softmax exp split between the Scalar engine (true Exp with the 1/8
    scale and a -2 shift fused) and the Vector engine (Schraudolph int16
    bit-trick straight from PSUM, debiased; bits read back as bf16).
    The -2 shift cancels in normalization.
  - Everything AllGathered (no redundant full-sequence local projections):
    chunk 0 = pair 0 lands while Q/K/V shard projections keep the PE busy.
  - V is AllGathered in [128x64] head-blocks so attention-time V loads are
    single contiguous descriptors.
  - Normalization: Z scattered to [128, 8] so the DVE reciprocal uses all
    lanes, DRAM stride-0 broadcast, multiply on GpSimd.
  - Output projection in two 384-wide chunks with the scalar engine doing
    final evictions.
"""

import math

import numpy as np


def _ensure_paths():
    try:
        import concourse  # noqa: F401
    except ImportError:
        import sys

        for p in ("/opt/trn_rl_repo", "/root/.axon_site/_ro/trn_rl_repo"):
            if p not in sys.path:
                sys.path.append(p)


_ensure_paths()

N_HEADS = 12
D_MODEL = 768
DH = 64
B = 1
S = 4096
N_CORES = 8
P = 128
LOG2E = 1.4426950408889634
ESHIFT = 2.0
SCHRAUD_DEBIAS = 2.0 - 1.0 / math.log(2.0) - 0.5


def install_ntff_hook():
    import sys
    import types

    try:
        from antenv.axon_hooks import get_axon_ntff_profile_hook  # noqa: F401

        return True
    except ImportError:
        pass
    try:
        import antenv
        from trn_agent_boot.trn_boot import _ntff_profile_via_ctypes

        hook = _ntff_profile_via_ctypes("/opt/axon/libaxon_pjrt.so")
        if hook is None:
            return False
        mod = types.ModuleType("antenv.axon_hooks")
        mod._hook = hook

        def set_axon_ntff_profile_hook(h):
            mod._hook = h

        def get_axon_ntff_profile_hook():
            return mod._hook

        mod.set_axon_ntff_profile_hook = set_axon_ntff_profile_hook
        mod.get_axon_ntff_profile_hook = get_axon_ntff_profile_hook
        sys.modules["antenv.axon_hooks"] = mod
        antenv.axon_hooks = mod
        return True
    except Exception:
        return False


def build_attention_nc(s_total=S, n_cores=N_CORES, n_heads=N_HEADS, dh=DH,
                       d_model=D_MODEL, scalar_frac=11):
    """scalar_frac/16 of exp k-tiles run on Scalar, rest on DVE."""
    import concourse.bass as bass  # noqa: F401
    import concourse.mybir as mybir
    import concourse.tile as tile
    from concourse import bacc

    dt = mybir.dt
    BF = dt.bfloat16
    F32 = dt.float32
    I16 = dt.int16
    EXP = mybir.ActivationFunctionType.Exp
    IDN = mybir.ActivationFunctionType.Identity
    CPY = mybir.ActivationFunctionType.Copy
    MUL = mybir.AluOpType.mult
    ADD = mybir.AluOpType.add

    HD = n_heads * dh
    assert HD == d_model
    SQ = s_total // n_cores       # 512
    NK = d_model // P             # 6 contraction tiles
    NPAIR = n_heads // 2          # 6
    NT = s_total // P             # 32 key tiles per pair
    NSQT = SQ // P                # 4
    VW = 160                      # vt row: [A 64 | 1 | pad | B 64 | 1 | pad]
    NVS = 8
    CHW = 512
    NCH = s_total // CHW
    e_scale = 1.0 / 8.0
    d_scale = P * LOG2E / 8.0     # schraudolph for bf16 bits
    d_bias = P * (127.0 - ESHIFT * LOG2E - SCHRAUD_DEBIAS)

    nc = bacc.Bacc("TRN2", target_bir_lowering=False, debug=False,
                   num_devices=n_cores)

    xt = nc.dram_tensor("xt", [P, NK, SQ], BF, kind="ExternalInput")
    xf = nc.dram_tensor("xf", [P, NK, s_total], BF, kind="ExternalInput")
    wq = nc.dram_tensor("wq", [P, NK, HD], BF, kind="ExternalInput")
    wk = nc.dram_tensor("wk", [P, NK, HD], BF, kind="ExternalInput")
    wv = nc.dram_tensor("wv", [P, NK, HD], BF, kind="ExternalInput")
    wo = nc.dram_tensor("wo", [P, NPAIR, d_model], BF, kind="ExternalInput")
    bqc = nc.dram_tensor("bqc", [P, NPAIR], F32, kind="ExternalInput")
    bo = nc.dram_tensor("bo", [1, d_model], BF, kind="ExternalInput")
    out = nc.dram_tensor("out", [SQ, d_model], F32, kind="ExternalOutput")

    def exp_on_scalar(idx):
        return (idx * scalar_frac) % 16 < scalar_frac

    with tile.TileContext(nc) as tc:
        from contextlib import ExitStack

        with ExitStack() as ctx:
            const = ctx.enter_context(tc.tile_pool(name="const", bufs=1))
            io = ctx.enter_context(tc.tile_pool(name="io", bufs=3))
            vio = ctx.enter_context(tc.tile_pool(name="vio", bufs=6))
            ps = ctx.enter_context(
                tc.tile_pool(name="ps", bufs=3, space="PSUM"))
            psY = ctx.enter_context(
                tc.tile_pool(name="psY", bufs=1, space="PSUM"))
            dram = ctx.enter_context(
                tc.tile_pool(name="dram", bufs=1, space="DRAM"))

            # ---------------- constants & weights ----------------
            ones_bf = const.tile([1, P], BF, tag="ones")
            nc.vector.memset(ones_bf[:], 1.0)
            biasm = const.tile([P, 1], F32, tag="biasm")
            nc.vector.memset(biasm[:], -ESHIFT)

            # producer-side V eviction slots with a baked ones column;
            # the ones ride through the AllGather inside each [128,65] block
            vev = const.tile([P, NVS, dh + 1], BF, tag="vev")
            for s in range(NVS):
                nc.vector.memset(vev[:, s, dh:dh + 1], 1.0)
            vev_i = [0]

            xt_sb = const.tile([P, NK, SQ], BF, tag="xt_sb")
            wq_sb = const.tile([P, NK, HD], BF, tag="wq_sb")
            wk_sb = const.tile([P, NK, HD], BF, tag="wk_sb")
            wv_sb = const.tile([P, NK, HD], BF, tag="wv_sb")
            nc.sync.dma_start(xt_sb[:], xt[:, :, :])
            nc.sync.dma_start(wk_sb[:], wk[:, :, :])
            nc.sync.dma_start(wv_sb[:], wv[:, :, :])
            nc.sync.dma_start(wq_sb[:], wq[:, :, :])
            wo_sb = const.tile([P, NPAIR, d_model], BF, tag="wo_sb")
            nc.sync.dma_start(wo_sb[:], wo[:, :, :])
            bq_sb = const.tile([P, NPAIR], F32, tag="bq_sb")
            nc.sync.dma_start(bq_sb[:], bqc[:, :])
            bo_sb = const.tile([1, d_model], BF, tag="bo_sb")
            nc.sync.dma_start(bo_sb[:], bo[:, :])

            qsb = const.tile([P, NPAIR, SQ], BF, tag="qsb")
            zbs_all = const.tile([P, NPAIR, SQ], F32, tag="zbs_all")
            y_sb = const.tile([P, NPAIR, SQ], BF, tag="y_sb")
            ynorm = const.tile([P, NPAIR, SQ], BF, tag="ynorm")

            kl = const.tile([P, NCH, CHW], BF, tag="kl")
            vlv = const.tile([P, NCH, 4, 2, dh + 1], BF, tag="vlv")
            for c in range(NCH):
                nc.vector.memset(vlv[:, c, :, :, dh:dh + 1], 1.0)

            scr = const.tile([1, 8], F32, tag="scr")
            nc.scalar.activation(scr[:], ones_bf[:, 0:8], EXP)

            # ---------------- DRAM collective tiles ----------------
            rg = [list(range(n_cores))]
            CH_PAIRS = [(1, 2), (3, 3)]
            kag, vag = {}, {}
            for ci, (p0, np_) in enumerate(CH_PAIRS):
                cw = np_ * P
                kb = dram.tile([cw, SQ], BF, tag=f"kb{ci}")
                ka = dram.tile([n_cores * cw, SQ], BF, tag=f"kag{ci}",
                               addr_space="Shared")
                kag[ci] = (kb, ka)
                vb = dram.tile([np_ * 2, P, NSQT * (dh + 1)], BF,
                               tag=f"vb{ci}")
                va = dram.tile([n_cores * np_ * 2, P, NSQT * (dh + 1)], BF,
                               tag=f"vag{ci}", addr_space="Shared")
                vag[ci] = (vb, va)
            pair2ch = {}
            for ci, (p0, np_) in enumerate(CH_PAIRS):
                for pl in range(np_):
                    pair2ch[p0 + pl] = (ci, pl)

            # ---------------- shard projections ----------------
            def proj_pair(w_sb, cs):
                pt = ps.tile([P, 2, SQ], F32, tag="sc")
                o = pt[:, 0, :]
                for k in range(NK):
                    nc.tensor.matmul(o, lhsT=w_sb[:, k, cs:cs + P],
                                     rhs=xt_sb[:, k, :],
                                     start=(k == 0), stop=(k == NK - 1))
                return pt

            def vproj_stile(s_, cs, width):
                pt = ps.tile([P, 2, SQ], F32, tag="sc")
                o = pt[:, 0, 0:width]
                rs = s_ * P
                for k in range(NK):
                    nc.tensor.matmul(
                        o, lhsT=xt_sb[:, k, rs:rs + P],
                        rhs=wv_sb[:, k, cs:cs + width],
                        start=(k == 0), stop=(k == NK - 1))
                return pt

            def emit_chunk(ci):
                p0, np_ = CH_PAIRS[ci]
                kb, ka = kag[ci]
                for pl in range(np_):
                    pt = proj_pair(wk_sb, (p0 + pl) * P)
                    k16 = io.tile([P, SQ], BF, tag="k16")
                    nc.scalar.activation(k16[:], pt[:, 0, :], CPY)
                    nc.scalar.dma_start(kb[pl * P:(pl + 1) * P, :], k16[:])
                nc.gpsimd.collective_compute(
                    "AllGather", mybir.AluOpType.bypass, replica_groups=rg,
                    ins=[kb.opt()], outs=[ka.opt()])
                vb, va = vag[ci]
                cw = np_ * P
                for s_ in range(NSQT):
                    pt = vproj_stile(s_, p0 * P, cw)
                    for h2 in range(np_ * 2):
                        sl = vev_i[0] % NVS
                        vev_i[0] += 1
                        if h2 % 2 == 0:
                            nc.scalar.activation(
                                vev[:, sl, 0:dh],
                                pt[:, 0, h2 * dh:(h2 + 1) * dh], CPY)
                        else:
                            nc.vector.tensor_copy(
                                vev[:, sl, 0:dh],
                                pt[:, 0, h2 * dh:(h2 + 1) * dh])
                        nc.scalar.dma_start(
                            vb[h2, :, s_ * (dh + 1):(s_ + 1) * (dh + 1)],
                            vev[:, sl, :])
                nc.gpsimd.collective_compute(
                    "AllGather", mybir.AluOpType.bypass, replica_groups=rg,
                    ins=[vb.opt()], outs=[va.opt()])

            emit_chunk(0)
            for p in range(NPAIR):
                pt = proj_pair(wq_sb, p * P)
                nc.scalar.activation(qsb[:, p, :], pt[:, 0, :], IDN,
                                     bias=bq_sb[:, p:p + 1], scale=1.0)
            emit_chunk(1)

            # ---------------- attention ----------------
            yA = psY.tile([dh + 1, SQ], F32, tag="yA")
            yB = psY.tile([dh + 1, SQ], F32, tag="yB")
            exp_idx = [0]

            def emit_attnv(pend, last):
                at16, vst, tl, t = pend
                for h in range(2):
                    yT = yA if h == 0 else yB
                    nc.tensor.matmul(
                        yT[:], lhsT=vst[:, h, tl, :],
                        rhs=at16[:, h, :],
                        start=(t == 0), stop=last)

            def finish_pair(p):
                zrt = io.tile([dh + 1, 2, SQ], F32, tag="zrt")
                nc.vector.tensor_copy(zrt[dh:dh + 1, 0, :], yA[dh:dh + 1, :])
                nc.vector.tensor_copy(zrt[dh:dh + 1, 1, :], yB[dh:dh + 1, :])
                zt = io.tile([P, 8], F32, tag="zt")
                nc.scalar.dma_start(zt[:], zrt[dh:dh + 1, :, :])
                zr = io.tile([P, 8], F32, tag="zr")
                nc.vector.reciprocal(zr[:], zt[:])
                zd = dram.tile([2, SQ], F32, tag=f"zd{p}")
                nc.scalar.dma_start(zd[:, :], zr[:])
                nc.vector.tensor_copy(y_sb[0:dh, p, :], yA[0:dh, :])
                ybt = io.tile([dh, SQ], BF, tag="ybt")
                nc.vector.tensor_copy(ybt[:], yB[0:dh, :])
                nc.scalar.dma_start(y_sb[dh:2 * dh, p, :], ybt[:])
                nc.scalar.dma_start(zbs_all[0:dh, p, :],
                                    zd[0:1, :].to_broadcast((dh, SQ)))
                nc.scalar.dma_start(zbs_all[dh:2 * dh, p, :],
                                    zd[1:2, :].to_broadcast((dh, SQ)))

            # ---- pair 0: local full-sequence K/V from xf ----
            pend = []
            for c8 in range(NCH):
                xfc = io.tile([P, NK, CHW], BF, tag="xfc")
                nc.sync.dma_start(xfc[:],
                                  xf[:, :, c8 * CHW:(c8 + 1) * CHW])
                pt = ps.tile([P, 2, SQ], F32, tag="sc")
                for k in range(NK):
                    nc.tensor.matmul(pt[:, 0, :], lhsT=wk_sb[:, k, 0:P],
                                     rhs=xfc[:, k, :],
                                     start=(k == 0), stop=(k == NK - 1))
                nc.vector.tensor_copy(kl[:, c8, :], pt[:, 0, :])
                for tt in range(4):
                    pv = ps.tile([P, 2, SQ], F32, tag="sc")
                    o = pv[:, 0, 0:P]
                    for k in range(NK):
                        nc.tensor.matmul(
                            o, lhsT=xfc[:, k, tt * P:(tt + 1) * P],
                            rhs=wv_sb[:, k, 0:P],
                            start=(k == 0), stop=(k == NK - 1))
                    nc.scalar.activation(vlv[:, c8, tt, 0, 0:dh],
                                         pv[:, 0, 0:dh], CPY)
                    nc.vector.tensor_copy(vlv[:, c8, tt, 1, 0:dh],
                                          pv[:, 0, dh:2 * dh])
                for tt in range(4):
                    t = c8 * 4 + tt
                    sc = ps.tile([P, 2, SQ], F32, tag="sc")
                    for h in range(2):
                        nc.tensor.matmul(
                            sc[:, h, :],
                            lhsT=kl[h * dh:(h + 1) * dh, c8,
                                    tt * P:(tt + 1) * P],
                            rhs=qsb[h * dh:(h + 1) * dh, 0, :],
                            start=True, stop=True,
                            tile_position=(h * dh, 0))
                    if len(pend) >= 2:
                        at16o, vsto, tlo, to = pend.pop(0)
                        for h in range(2):
                            yT = yA if h == 0 else yB
                            nc.tensor.matmul(
                                yT[:], lhsT=vlv[:, to // 4, to % 4, h, :],
                                rhs=at16o[:, h, :],
                                start=(to == 0), stop=False)
                    at16 = vio.tile([P, 2, SQ], BF, tag="at16")
                    i = exp_idx[0]
                    exp_idx[0] += 1
                    if exp_on_scalar(i):
                        nc.scalar.activation(at16[:], sc[:], EXP,
                                             bias=biasm[:], scale=e_scale)
                    else:
                        nc.vector.tensor_scalar(at16[:].bitcast(I16), sc[:],
                                                d_scale, d_bias, MUL, ADD)
                    pend.append((at16, None, tt, t))
            for pi, (at16o, _, _, to) in enumerate(pend):
                for h in range(2):
                    yT = yA if h == 0 else yB
                    nc.tensor.matmul(
                        yT[:], lhsT=vlv[:, to // 4, to % 4, h, :],
                        rhs=at16o[:, h, :],
                        start=(to == 0), stop=(pi == len(pend) - 1))
            pend = []
            finish_pair(0)
            ynorm_q = [0]

            for p in range(1, NPAIR):
                ci, pl = pair2ch[p]
                cw = CH_PAIRS[ci][1] * P
                n2 = CH_PAIRS[ci][1] * 2
                ka = kag[ci][1]
                va = vag[ci][1]
                pend = []
                for r in range(n_cores):
                    ktp = vio.tile([P, SQ], BF, tag="ktp")
                    nc.gpsimd.dma_start(
                        ktp[:], ka[r * cw + pl * P:r * cw + (pl + 1) * P, :])
                    # all of this rank's V for the pair: one fat DMA into
                    # contiguous staging (fat 520B descriptors)
                    vst = vio.tile([P, 2, NSQT, dh + 1], BF, tag="vst")
                    nc.gpsimd.dma_start(
                        vst[:],
                        va[r * n2 + 2 * pl:r * n2 + 2 * pl + 2, :, :]
                        .transpose([1, 0, 2]))
                    for tr in range(NSQT):
                        t = r * NSQT + tr
                        sc = ps.tile([P, 2, SQ], F32, tag="sc")
                        for h in range(2):
                            nc.tensor.matmul(
                                sc[:, h, :],
                                lhsT=ktp[h * dh:(h + 1) * dh,
                                         tr * P:(tr + 1) * P],
                                rhs=qsb[h * dh:(h + 1) * dh, p, :],
                                start=True, stop=True,
                                tile_position=(h * dh, 0))
                        if len(pend) >= 2:
                            emit_attnv(pend.pop(0), False)
                        at16 = vio.tile([P, 2, SQ], BF, tag="at16")
                        i = exp_idx[0]
                        exp_idx[0] += 1
                        if exp_on_scalar(i):
                            nc.scalar.activation(at16[:], sc[:], EXP,
                                                 bias=biasm[:],
                                                 scale=e_scale)
                        else:
                            nc.vector.tensor_scalar(
                                at16[:].bitcast(I16), sc[:],
                                d_scale, d_bias, MUL, ADD)
                        pend.append((at16, vst, tr, t))
                while len(pend) > 1:
                    emit_attnv(pend.pop(0), False)
                emit_attnv(pend.pop(0), True)
                finish_pair(p)

            # remaining normalization multiplies, split across the two
            # idle-by-now engines so the output projection starts sooner
            for p in range(ynorm_q[0], NPAIR):
                eng = nc.gpsimd if p % 2 == 0 else nc.vector
                eng.tensor_mul(out=ynorm[:, p, :],
                               in0=y_sb[:, p, :],
                               in1=zbs_all[:, p, :])

            # ---------------- output projection ----------------
            DMC = 384
            for s_ in range(NSQT):
                rs = s_ * P
                pt = ps.tile([P, 2, SQ], F32, tag="sc")
                for ch in range(2):
                    o = pt[:, ch, 0:DMC]
                    c0 = ch * DMC
                    for p in range(NPAIR):
                        nc.tensor.matmul(o, lhsT=ynorm[:, p, rs:rs + P],
                                         rhs=wo_sb[:, p, c0:c0 + DMC],
                                         start=(p == 0), stop=False)
                    nc.tensor.matmul(o, lhsT=ones_bf[:, 0:P],
                                     rhs=bo_sb[:, c0:c0 + DMC],
                                     start=False, stop=True)
                osb = io.tile([P, 2, DMC], F32, tag="osb")
                nc.scalar.activation(osb[:], pt[:, :, 0:DMC], CPY)
                nc.sync.dma_start(out[rs:rs + P, :], osb[:])

    nc.compile()
    return nc


# ---------------------------------------------------------------------------
_CACHE = {}


def _get_nc():
    if "nc" not in _CACHE:
        _CACHE["nc"] = build_attention_nc()
    return _CACHE["nc"]


def _fold6(a):
    """[768, N] -> [128, 6, N]: row = 128k + p."""
    n = a.shape[1]
    return np.ascontiguousarray(a.reshape(6, 128, n).transpose(1, 0, 2))


def make_in_maps(x, Wq, bq, Wk, bk, Wv, bv, Wo, bo, n_cores=N_CORES):
    import ml_dtypes

    bf = ml_dtypes.bfloat16
    sq = x.shape[1] // n_cores
    x2 = np.asarray(x, np.float32).reshape(x.shape[1], D_MODEL)
    xT = x2.T

    Wo_f = np.asarray(Wo, np.float32)
    shared = {
        "wq": _fold6(np.asarray(Wq, np.float32)).astype(bf),
        "wk": _fold6(np.asarray(Wk, np.float32)).astype(bf),
        "wv": _fold6(np.asarray(Wv, np.float32)).astype(bf),
        "wo": np.ascontiguousarray(
            Wo_f.reshape(N_HEADS // 2, P, D_MODEL).transpose(1, 0, 2)
        ).astype(bf),
        "bqc": np.ascontiguousarray(
            np.asarray(bq, np.float32).reshape(N_HEADS // 2, P).T),
        "bo": (np.asarray(bo, np.float32)
               + np.asarray(bv, np.float32) @ Wo_f).reshape(1, -1).astype(bf),
    }
    shared["xf"] = _fold6(xT).astype(bf)
    in_maps = []
    for c in range(n_cores):
        xtc = _fold6(
            np.ascontiguousarray(xT[:, c * sq:(c + 1) * sq])).astype(bf)
        in_maps.append({"xt": xtc, **shared})
    return in_maps


def kernel(x, Wq, bq, Wk, bk, Wv, bv, Wo, bo):
    from concourse.bass_utils import run_bass_kernel_spmd

    nc = _get_nc()
    in_maps = make_in_maps(x, Wq, bq, Wk, bk, Wv, bv, Wo, bo)
    res = run_bass_kernel_spmd(nc, in_maps, core_ids=list(range(N_CORES)))
    out = np.concatenate([res.results[c]["out"] for c in range(N_CORES)],
                         axis=0)
    return out.reshape(B, S, D_MODEL).astype(np.float32)
